# revision 54
# baseline (speedup 1.0000x reference)
"""Grouped-query attention (2 query heads, 1 pooled KV head) with RoPE,
causal softmax — Trainium2 Bass/Tile kernel, 8 NeuronCores.

Sharding: one core per (batch, head) pair (4 x 2 = 8 cores). The pooled KV
head is head-averaged on the host (mean over heads commutes with the linear
projection), so each core does: q/k/v projections, RoPE on q/k, causal
attention. ~97% PE occupancy at the bf16 matmul roofline.

Design (all matmul operands bf16 — 1 cycle/row on the PE vs 4 for fp32;
accumulation stays fp32 in PSUM; 2e-2 tolerance leaves 4x margin):

phase 1 — projections + RoPE:
  - q and k are projected DIRECTLY in transposed [d, t] layout (weight
    chunk stationary, xT slab moving), so the scores matmul needs no PE
    transposes at all. v is projected in natural [t, d] (x slice
    stationary).
  - RoPE in [d, t]: the head dim (partitions) is permuted to
    evens-then-odds, making the pair swap a partition-block swap between
    d-tiles (0<->2, 1<->3); sin sign is baked into the table; the
    permutation cancels in the q.k^T contraction. Runs on DVE off the
    critical path.
  - PSUM: one bank per dt chain (8 total); each chain's PSUM->SBUF bf16
    cast (Activation) overlaps the next chain's matmuls, and the v chains
    reuse the q banks (same pool tag). Slab 0 runs c-major so the PE
    consumes the startup x/wq chunk stream as it lands; dummy matmuls on
    unwritten SBUF warm the PE p-state ramp during the initial DMA wait.

phase 2 — causal attention at minimal 128x128 causal coverage (136/256
tile pairs):
  - scoresT [s, t-tile] accumulated over 4 d-chunks; up to 4 s-tiles
    batched per PSUM bank (bank-zero by the group's first start=True) so
    ONE exp op serves 4 s-tiles and ScalarE stays well ahead of the PE.
  - exp fused with the 1/sqrt(D) scale; causal mask = bf16 multiply on
    the diagonal tile only; denominators are [128,1] columns from tiny
    matmuls against a ones vector, accumulated start=False into a shared
    memset-managed bank; AV in [t, d] (expt stationary, v moving).
  - a 3-deep software pipeline (sums/AV consumers run 3 s-tile groups
    behind the scores chain, across slab boundaries) hides the exp/mask
    latency; normalization is a per-partition-scalar multiply on DVE and
    the output leaves natural [T, D] in bf16 (host upcasts).
  - one PSUM pool for the whole kernel: phase 2 reuses phase 1's bank
    tags, so the cross-phase waits are per-bank (all satisfied early)
    instead of a pool-close barrier that would idle the PE.
"""

import sys

sys.path.insert(0, "/opt/trn_rl_repo")

import numpy as np
import ml_dtypes

BF16 = ml_dtypes.bfloat16

B, T, C = 4, 2048, 2048
H, D = 2, 512
NCORES = 8
ROPE_THETA = 10000.0
P = 128
NT = T // P  # 16 t/s tiles of 128
NCC = C // P  # 16 contraction chunks of 128
NDT = D // P  # 4 head-dim tiles of 128
TS = 512  # t-slab width (both phases)
NSLAB = T // TS  # 4

_CACHE = {}


def _build():
    import concourse.bass as bass
    import concourse.mybir as mybir
    from concourse import bacc
    from concourse.tile import TileContext

    fp32 = mybir.dt.float32
    bf16 = mybir.dt.bfloat16
    EXP = mybir.ActivationFunctionType.Exp
    COPY = mybir.ActivationFunctionType.Copy

    nc = bacc.Bacc()
    xT = nc.dram_tensor("xT", [C, T], bf16, kind="ExternalInput")
    wq = nc.dram_tensor("wq", [C, D], bf16, kind="ExternalInput")  # pi-permuted
    wk = nc.dram_tensor("wk", [C, D], bf16, kind="ExternalInput")  # pi-perm, head-avg
    wv = nc.dram_tensor("wv", [C, D], bf16, kind="ExternalInput")  # head-avg
    cosT = nc.dram_tensor("cosT", [P, 2, T], bf16, kind="ExternalInput")
    sinT = nc.dram_tensor("sinT", [P, 4, T], bf16, kind="ExternalInput")
    masks = nc.dram_tensor("masks", [P, P], bf16, kind="ExternalInput")
    ones = nc.dram_tensor("ones", [P, 1], bf16, kind="ExternalInput")
    o = nc.dram_tensor("o", [T, D], bf16, kind="ExternalOutput")

    scale = float(D) ** -0.5

    LA = 3  # lookahead (in s-tile groups) of the phase-2 software pipeline

    with TileContext(nc) as tc:
        with (
            tc.tile_pool(name="persist", bufs=1) as pp,
            # one PSUM pool for the whole kernel: phase 2 reuses phase 1's
            # bank tags, so cross-phase waits are per-bank (all satisfied
            # early) instead of a pool-close barrier that idles the PE
            tc.tile_pool(name="ps", bufs=1, space="PSUM") as ps1,
            tc.tile_pool(name="expp", bufs=LA + 2) as ep,
            tc.tile_pool(name="outp", bufs=3) as op_,
        ):
            qT_sb = pp.tile([P, NDT, T], bf16)
            kT_sb = pp.tile([P, NDT, T], bf16)
            v_sb = pp.tile([P, NT, D], bf16)
            ones_sb = op_.tile([P, 1], bf16, tag="ones", bufs=1)
            mask_sb = op_.tile([P, P], bf16, tag="masks", bufs=1)

            # ---------------- phase 1: projections + rope ------------------
            with (
                tc.tile_pool(name="wpool", bufs=1) as wp,
                tc.tile_pool(name="stream", bufs=2) as sp,
                tc.tile_pool(name="casts", bufs=2) as cp,
                tc.tile_pool(name="rope", bufs=2) as rp,
            ):
                wq_sb = wp.tile([P, NCC, D], bf16)
                wk_sb = wp.tile([P, NCC, D], bf16)
                wv_sb = wp.tile([P, NCC, D], bf16)
                cos_sb = wp.tile([P, 2, T], bf16)
                sin_sb = wp.tile([P, 4, T], bf16)
                xs0 = sp.tile([P, NCC, TS], bf16, tag="x", name="xs0")
                # startup: stream slab-0 x and wq in interleaved 4-chunk
                # groups so the first matmul chain starts ~3.5us in; wk next
                # (needed when the k chains start), cos/sin before wv (RoPE
                # on DVE stalls harmlessly; the v matmuls gate on wv).
                wq_r = wq.rearrange("(cc p) d -> p cc d", p=P)
                wk_r = wk.rearrange("(cc p) d -> p cc d", p=P)
                wv_r = wv.rearrange("(cc p) d -> p cc d", p=P)
                xT_r = xT.rearrange("(cc p) t -> p cc t", p=P)
                for g0, g1 in (
                    (0, 2), (2, 4), (4, 6), (6, 8),
                    (8, 10), (10, 12), (12, 14), (14, 16),
                ):
                    gs = slice(g0, g1)
                    nc.sync.dma_start(out=xs0[:, gs, :], in_=xT_r[:, gs, 0:TS])
                    nc.sync.dma_start(out=wq_sb[:, gs, :], in_=wq_r[:, gs, :])
                G = 4
                for g in range(NCC // G):
                    gs = slice(g * G, (g + 1) * G)
                    nc.sync.dma_start(out=wk_sb[:, gs, :], in_=wk_r[:, gs, :])
                nc.sync.dma_start(out=wv_sb, in_=wv_r)
                nc.sync.dma_start(out=cos_sb, in_=cosT[:, :, :])
                nc.sync.dma_start(out=sin_sb, in_=sinT[:, :, :])
                nc.sync.dma_start(out=ones_sb, in_=ones[:, :])
                nc.sync.dma_start(out=mask_sb, in_=masks[:, :])

                # warm up the PE clock during the startup DMA wait: dummy
                # matmuls on unwritten SBUF (values irrelevant; the bank is
                # reset by the first real start=True chain). The p-state
                # ramp (0.65/1.2 GHz until 3us continuously busy) then
                # completes before real data lands.
                warm = ps1.tile([P, TS], fp32, tag="ps_q0", name="warm")
                for w in range(5):
                    nc.tensor.matmul(
                        warm,
                        qT_sb[:, NDT - 1, T - P : T],
                        kT_sb[:, NDT - 1, T - TS : T],
                        start=(w == 0),
                        stop=(w == 4),
                        skip_group_check=True,
                    )

                for sl in range(NSLAB):
                    t0 = sl * TS
                    tsl = slice(t0, t0 + TS)
                    if sl == 0:
                        xs = xs0
                    else:
                        xs = sp.tile([P, NCC, TS], bf16, tag="x", name="xs")
                        nc.sync.dma_start(out=xs, in_=xT_r[:, :, tsl])
                    # q then k, one PSUM bank per dt chain. Slab 0 runs the
                    # chains c-major so the PE consumes x/w chunk groups as
                    # they stream in; later slabs run dt-major so each
                    # chain's bf16 cast overlaps the next chain's matmuls.
                    qc = [None] * NDT
                    kc = [None] * NDT
                    for w_sb, cc in ((wq_sb, qc), (wk_sb, kc)):
                        tg = "q" if cc is qc else "k"
                        pts = [
                            ps1.tile([P, TS], fp32, tag=f"ps_{tg}{dt}", name="pt")
                            for dt in range(NDT)
                        ]
                        if sl == 0:
                            for c in range(NCC):
                                for dt in range(NDT):
                                    nc.tensor.matmul(
                                        pts[dt],
                                        w_sb[:, c, dt * P : (dt + 1) * P],
                                        xs[:, c, :],
                                        start=(c == 0),
                                        stop=(c == NCC - 1),
                                    )
                        else:
                            for dt in range(NDT):
                                for c in range(NCC):
                                    nc.tensor.matmul(
                                        pts[dt],
                                        w_sb[:, c, dt * P : (dt + 1) * P],
                                        xs[:, c, :],
                                        start=(c == 0),
                                        stop=(c == NCC - 1),
                                    )
                        for dt in range(NDT):
                            cc[dt] = cp.tile(
                                [P, TS], bf16, tag=f"c_{tg}{dt}", name=f"c_{tg}{dt}"
                            )
                            nc.scalar.activation(out=cc[dt], in_=pts[dt], func=COPY)
                    # RoPE in [d, t]: rot[dt] = cc[dt]*cos[dt%2] + cc[dt^2]*sin[dt]
                    for cc, dst in ((qc, qT_sb), (kc, kT_sb)):
                        for dt in range(NDT):
                            tmp = rp.tile([P, TS], bf16, tag="tmp")
                            nc.vector.tensor_mul(
                                tmp, cc[(dt + 2) % NDT], sin_sb[:, dt, tsl]
                            )
                            rot = rp.tile([P, TS], bf16, tag="rot")
                            nc.vector.tensor_mul(rot, cc[dt], cos_sb[:, dt % 2, tsl])
                            nc.vector.tensor_add(dst[:, dt, tsl], rot, tmp)
                    # v: natural [t, d]; reuses the q banks (same tags, bufs=1)
                    for tt in range(TS // P):
                        pt = ps1.tile([P, D], fp32, tag=f"ps_q{tt}")
                        for c in range(NCC):
                            nc.tensor.matmul(
                                pt,
                                xs[:, c, tt * P : (tt + 1) * P],
                                wv_sb[:, c, :],
                                start=(c == 0),
                                stop=(c == NCC - 1),
                            )
                        nc.scalar.activation(
                            out=v_sb[:, sl * (TS // P) + tt, :], in_=pt, func=COPY
                        )

            # ---------------- phase 2: causal attention -------------------
            # 128-wide t-slabs (one t-tile per slab) for minimal causal
            # coverage (136 of 256 s,t tile pairs). Each step is only ~430ns
            # of PE work vs ~1us of exp/mask latency, so the sums/AV
            # consumers of step n are emitted 3 steps behind the scores
            # chain of step n (a software pipeline that also runs across
            # slab boundaries).
            if True:
                # PSUM bank tags, reusing phase 1's: av(j) cycles the four
                # ps_q banks (drained by the v-copies), sums(j) alternates
                # ps_k0/1, the sc groups alternate ps_k2/3 (drained by the
                # kc copies) — no cross-phase pool barrier anywhere.
                state = {}  # j -> (sums, av)
                GW = 4  # s-tiles batched per PSUM bank / exp op
                ngrp = [0]
                # both slabs-in-flight share ONE denominator bank (two
                # [P,1] columns, zeroed by memset instead of matmul
                # bank-clears), freeing a third rotating bank for sc
                sums2 = ps1.tile([P, TS], fp32, tag="ps_k0", name="sums2")
                nc.vector.memset(sums2[:, 0:2], 0.0)

                def consume(j, grp, expt):
                    # sums column + AV consumers of an s-tile group of slab j
                    if grp[0] == 0:
                        state[j] = (
                            sums2[:, j % 2 : j % 2 + 1],
                            ps1.tile([P, D], fp32, tag=f"ps_q{j % 4}", name="av"),
                        )
                    sums, av = state[j]
                    final = grp[-1] == j
                    if final:
                        # slab-final group: all sums matmuls first so the
                        # reciprocal overlaps the AV matmuls
                        for i, st in enumerate(grp):
                            nc.tensor.matmul(
                                sums,
                                expt[:, i * P : (i + 1) * P],
                                ones_sb,
                                start=False,
                                stop=(st == j),
                                skip_group_check=True,
                            )
                        rec = op_.tile([P, 1], fp32, tag="rec", bufs=2)
                        nc.vector.reciprocal(rec, sums)
                        nc.vector.memset(sums, 0.0)  # ready for slab j+2
                    for i, st in enumerate(grp):
                        lhs = expt[:, i * P : (i + 1) * P]
                        if not final:
                            nc.tensor.matmul(
                                sums, lhs, ones_sb, start=False, stop=False,
                                skip_group_check=True,
                            )
                        nc.tensor.matmul(
                            av, lhs, v_sb[:, st, :], start=(st == 0), stop=(st == j)
                        )
                    if final:  # normalize + store
                        r0 = j * P
                        ob = op_.tile([P, D], bf16, tag="ob", bufs=4)
                        # normalize on DVE so the act queue stays exp-only
                        # (act gates the sc-bank recycling)
                        nc.vector.tensor_scalar_mul(ob, av, rec)
                        nc.sync.dma_start(out=o[r0 : r0 + P, :], in_=ob)
                        del state[j]

                pending = []
                for j in range(NT):  # 16 slabs of 128
                    tsl = slice(j * P, (j + 1) * P)
                    for g0 in range(0, j + 1, GW):
                        grp = list(range(g0, min(g0 + GW, j + 1)))
                        W = len(grp) * P
                        sc = ps1.tile(
                            [P, GW * P], fp32, tag=f"ps_k{1 + ngrp[0] % 3}",
                            name="sc",
                        )
                        ngrp[0] += 1
                        for i, st in enumerate(grp):
                            for dt in range(NDT):
                                # start=True zeroes the whole bank, so only
                                # the group's first matmul sets it; later
                                # columns accumulate onto the cleared bank.
                                nc.tensor.matmul(
                                    sc[:, i * P : (i + 1) * P],
                                    kT_sb[:, dt, st * P : (st + 1) * P],
                                    qT_sb[:, dt, tsl],
                                    start=(i == 0 and dt == 0),
                                    stop=(dt == NDT - 1),
                                    skip_group_check=True,
                                )
                        expt = ep.tile([P, GW * P], bf16, tag="exp")
                        nc.scalar.activation(
                            out=expt[:, 0:W], in_=sc[:, 0:W], func=EXP, scale=scale
                        )
                        if grp[-1] == j:  # diagonal tile: zero the s > t half
                            off = (len(grp) - 1) * P
                            nc.vector.tensor_mul(
                                expt[:, off : off + P],
                                expt[:, off : off + P],
                                mask_sb,
                            )
                        pending.append((j, grp, expt))
                        if len(pending) > LA:
                            consume(*pending.pop(0))
                for it in pending:
                    consume(*it)

    nc.finalize()
    return nc


def _host_inputs(x, Wq, Wk, Wv):
    pi = np.concatenate([np.arange(0, D, 2), np.arange(1, D, 2)])

    wk_avg = Wk.mean(axis=0)  # [D, C]
    wv_avg = Wv.mean(axis=0)
    wk_p = np.ascontiguousarray(wk_avg.T[:, pi]).astype(BF16)
    wv_t = np.ascontiguousarray(wv_avg.T).astype(BF16)

    freqs = 1.0 / (ROPE_THETA ** (np.arange(0, D, 2, dtype=np.float64) / D))
    t = np.arange(T, dtype=np.float64)
    # cosT[r, g, t] = cos(t * f_{r+128g}); sinT sign baked: -sin for the
    # even half (g 0,1), +sin for the odd half (g 2,3).
    ang = t[None, :] * freqs[:, None]  # [256, T]
    cosT = np.cos(ang).reshape(2, P, T).transpose(1, 0, 2).astype(BF16)
    s = np.sin(ang).reshape(2, P, T).transpose(1, 0, 2)
    sinT = np.concatenate([-s, s], axis=1).astype(BF16)

    m = (np.arange(P)[:, None] <= np.arange(P)[None, :]).astype(np.float32)

    shared = {
        "wk": wk_p,
        "wv": wv_t,
        "cosT": np.ascontiguousarray(cosT),
        "sinT": np.ascontiguousarray(sinT),
        "masks": m.astype(BF16),
        "ones": np.ones((P, 1), BF16),
    }
    xTs = [np.ascontiguousarray(x[b].T).astype(BF16) for b in range(B)]
    wqs = [np.ascontiguousarray(Wq[h].T[:, pi]).astype(BF16) for h in range(H)]
    in_maps = []
    for i in range(NCORES):
        b, h = i // H, i % H
        in_maps.append(
            {
                "xT": xTs[b],
                "wq": wqs[h],
                **shared,
            }
        )
    return in_maps


def _run(x, Wq, Wk, Wv, trace=False):
    from concourse.bass_utils import run_bass_kernel_spmd

    if "nc" not in _CACHE:
        _CACHE["nc"] = _build()
    in_maps = _host_inputs(x, Wq, Wk, Wv)
    res = run_bass_kernel_spmd(
        _CACHE["nc"], in_maps, list(range(NCORES)), trace=trace
    )
    out = np.empty((B, H, T, D), np.float32)
    for i in range(NCORES):
        out[i // H, i % H] = res.results[i]["o"].astype(np.float32)
    return out.reshape(B, T, H * D), res


def kernel(**inputs):
    out, _ = _run(inputs["x"], inputs["Wq"], inputs["Wk"], inputs["Wv"])
    return out
